# revision 13
# baseline (speedup 1.0000x reference)
"""Trainium2 Bass kernel for nn_DynamicGraphLearner.

Computes, for full inputs (B=16, N=2048, D=64):
    adj_base = relu((emb @ w1.T + b1) @ (emb @ w2.T + b2).T)          [N, N]
    out      = softmax(adj_base + x xT + (v_i - v_j), axis=-1)        [B, N, N]
with v = x @ wp.T + wp_b.

Key algebraic simplifications (softmax is invariant to per-row shifts):
  * the +v_i term and the wp_b constant cancel entirely;
  * the -v_j term is linear in x_j, so it folds into the Gram matmul:
        logits_ij = adj_base_ij + (x_i - wp) . x_j
    i.e. the matmul lhs is simply (xT - wp) -- no separate bias pass.

Sharding: rows (the softmax i axis) are split 8 ways, 256 rows per core;
every core handles all 16 batches for its row slice, so the softmax stays
core-local and no collectives are needed.  adj_base is computed per core
only for its own 256-row slice.

Host-side marshaling: x_temp is passed transposed (xT [B, D, N] -> [B*D, N])
so the device reads clean contiguous 1 MiB tiles; two batches are packed per
SBUF tile (partitions 0:64 = batch 2q, 64:128 = batch 2q+1) and computed as
independent K=64 matmuls at partition bases 0 and 64.

Per [128, 2048] output tile:
  PE : 4 matmuls (512-col chunks) -> PSUM (dyn - v_j term), then 4
       identity-matmuls accumulating adj_base into the same PSUM banks
       (tensor_tensor_reduce is avoided: it wedges the device runtime)
  DVE: reduce_max over the PSUM tile (negated)
  ACT: exp(psum + (-max)) PSUM->SBUF with fused row-sum accumulation
  DVE: reciprocal, renormalize (tensor_scalar 2x mode)
  DMA: store 1 MiB contiguous
"""

import sys

if "/opt/trn_rl_repo" not in sys.path:
    sys.path.insert(0, "/opt/trn_rl_repo")

import numpy as np

import concourse.bass as bass
import concourse.tile as tile
from concourse import bacc, mybir
from concourse.bass_utils import run_bass_kernel_spmd

NCORES = 8
B, N, D = 16, 2048, 64
ROWS = N // NCORES  # 256 rows per core
FP = mybir.dt.float32

_NC_CACHE = {}


def _build_nc(reps=1):
    # reps>1 repeats the main loop (same outputs, idempotent) — used only by
    # the benchmark harness to amortize per-dispatch overhead out of timings.
    nc = bacc.Bacc(None)

    xt = nc.dram_tensor("xt", [B * D, N], FP, kind="ExternalInput")
    xt_rows = nc.dram_tensor("xt_rows", [B * D, ROWS], FP, kind="ExternalInput")
    embt = nc.dram_tensor("embt", [D, N], FP, kind="ExternalInput")
    embt_rows = nc.dram_tensor("embt_rows", [D, ROWS], FP, kind="ExternalInput")
    w1t = nc.dram_tensor("w1t", [D, D], FP, kind="ExternalInput")
    w2t = nc.dram_tensor("w2t", [D, D], FP, kind="ExternalInput")
    w1b = nc.dram_tensor("w1b", [D, 1], FP, kind="ExternalInput")
    w2b = nc.dram_tensor("w2b", [D, 1], FP, kind="ExternalInput")
    wp2 = nc.dram_tensor("wp2", [128, 1], FP, kind="ExternalInput")
    ident = nc.dram_tensor("ident", [128, 128], FP, kind="ExternalInput")
    out = nc.dram_tensor("out", [B * ROWS, N], FP, kind="ExternalOutput")

    Exp = mybir.ActivationFunctionType.Exp
    Relu = mybir.ActivationFunctionType.Relu
    Alu = mybir.AluOpType

    with tile.TileContext(nc) as tc:
        with (
            tc.tile_pool(name="const", bufs=1) as cpool,
            tc.tile_pool(name="ps", bufs=2, space="PSUM") as ps,
            tc.tile_pool(name="xp", bufs=3) as xpool,
            tc.tile_pool(name="op", bufs=3) as opool,
            tc.tile_pool(name="st", bufs=4) as spool,
        ):
            # ---- constants ----
            embt_sb = cpool.tile([D, N], FP)
            nc.scalar.dma_start(embt_sb[:], embt[:])
            embtr_sb = cpool.tile([D, ROWS], FP)
            nc.scalar.dma_start(embtr_sb[:], embt_rows[:])
            w1t_sb = cpool.tile([D, D], FP)
            nc.scalar.dma_start(w1t_sb[:], w1t[:])
            w2t_sb = cpool.tile([D, D], FP)
            nc.scalar.dma_start(w2t_sb[:], w2t[:])
            w1b_sb = cpool.tile([D, 1], FP)
            nc.scalar.dma_start(w1b_sb[:], w1b[:])
            w2b_sb = cpool.tile([D, 1], FP)
            nc.scalar.dma_start(w2b_sb[:], w2b[:])
            wp2_sb = cpool.tile([128, 1], FP)
            nc.scalar.dma_start(wp2_sb[:], wp2[:])
            id_sb = cpool.tile([128, 128], FP)
            nc.scalar.dma_start(id_sb[:], ident[:])

            # ---- node_1cT [64, 256] = w1 @ embT[:, rows] + b1 ----
            p1 = ps.tile([128, 2048], FP, tag="pm")
            nc.tensor.matmul(p1[0:D, 0:ROWS], w1t_sb[:], embtr_sb[:], start=True, stop=True)
            n1t_sb = cpool.tile([D, ROWS], FP)
            nc.vector.tensor_scalar_add(n1t_sb[:], p1[0:D, 0:ROWS], w1b_sb[:])

            # ---- node_2T [64, 2048] = w2 @ embT + b2 ----
            n2t_sb = cpool.tile([D, N], FP)
            p2 = ps.tile([128, 2048], FP, tag="pm")
            for c in range(4):
                nc.tensor.matmul(
                    p2[0:D, c * 512 : (c + 1) * 512],
                    w2t_sb[:],
                    embt_sb[:, c * 512 : (c + 1) * 512],
                    start=True,
                    stop=True,
                )
            nc.vector.tensor_scalar_add(n2t_sb[:], p2[0:D, :], w2b_sb[:])

            # ---- adj_base rows slice: 2 tiles of [128, 2048], relu'd ----
            adj_sb = []
            for rt in range(2):
                a = cpool.tile([128, N], FP, name=f"adj{rt}")
                adj_sb.append(a)
            for rt in range(2):
                pa = ps.tile([128, 2048], FP, tag="pm", name=f"pa_{rt}")
                for c in range(4):
                    nc.tensor.matmul(
                        pa[:, c * 512 : (c + 1) * 512],
                        n1t_sb[:, rt * 128 : (rt + 1) * 128],
                        n2t_sb[:, c * 512 : (c + 1) * 512],
                        start=True,
                        stop=True,
                    )
                nc.scalar.activation(adj_sb[rt][:], pa[:], Relu)

            # ---- main loop: 8 batch-pairs x 2 batches x 2 row tiles ----
            for rep, q in [(r, qq) for r in range(reps) for qq in range(NCORES)]:
                # packed pair of batches: partitions 0:64 = batch 2q, 64:128 = 2q+1
                xt_sb = xpool.tile([128, N], FP, tag="xt", name=f"xt{rep}_{q}")
                nc.scalar.dma_start(xt_sb[:], xt[q * 128 : (q + 1) * 128, :])
                xtr_sb = xpool.tile([128, ROWS], FP, tag="xtr", name=f"xtr{rep}_{q}")
                nc.scalar.dma_start(xtr_sb[:], xt_rows[q * 128 : (q + 1) * 128, :])
                # lhs = xT(rows) - wp  (folds the -v_j term into the matmul)
                lhs = xpool.tile([128, ROWS], FP, tag="lhs", name=f"lhs{rep}_{q}")
                nc.vector.tensor_scalar(
                    lhs[:], xtr_sb[:], wp2_sb[:], None, op0=Alu.subtract
                )

                for sb in range(2):  # sub-batch within the pair
                    base = D * sb
                    b = 2 * q + sb
                    for rt in range(2):  # row tile within the 256-row slice
                        o_sb = opool.tile([128, N], FP, tag="o", name=f"o{rep}_{q}_{sb}_{rt}")
                        nmax = spool.tile([128, 1], FP, tag="nm", name=f"nm{rep}_{b}_{rt}")
                        ssum = spool.tile([128, 1], FP, tag="ss", name=f"ss{rep}_{b}_{rt}")
                        rcp = spool.tile([128, 1], FP, tag="rc", name=f"rc{rep}_{b}_{rt}")

                        pm = ps.tile([128, N], FP, tag="pm", name=f"pm{rep}_{b}_{rt}")
                        for c in range(4):
                            nc.tensor.matmul(
                                pm[:, c * 512 : (c + 1) * 512],
                                lhs[base : base + D, rt * 128 : (rt + 1) * 128],
                                xt_sb[base : base + D, c * 512 : (c + 1) * 512],
                                start=True,
                                stop=False,
                            )
                        # accumulate adj_base into the same PSUM banks via
                        # identity matmul (I.T @ adj = adj, has_written add)
                        for c in range(4):
                            nc.tensor.matmul(
                                pm[:, c * 512 : (c + 1) * 512],
                                id_sb[:],
                                adj_sb[rt][:, c * 512 : (c + 1) * 512],
                                start=False,
                                stop=True,
                            )
                        # negated row max of the full logit tile
                        nc.vector.tensor_reduce(
                            nmax[:], pm[:], axis=mybir.AxisListType.X, op=Alu.max,
                            negate=True,
                        )
                        # exp(psum - max) -> SBUF, with fused row-sum
                        nc.scalar.activation(
                            o_sb[:], pm[:], Exp, bias=nmax[:], scale=1.0,
                            accum_out=ssum[:],
                        )
                        nc.vector.reciprocal(rcp[:], ssum[:])
                        nc.vector.tensor_scalar_mul(o_sb[:], o_sb[:], rcp[:])
                        row0 = b * ROWS + rt * 128
                        nc.sync.dma_start(out[row0 : row0 + 128, :], o_sb[:])

    nc.finalize()
    return nc


def _get_nc():
    if "nc" not in _NC_CACHE:
        _NC_CACHE["nc"] = _build_nc()
    return _NC_CACHE["nc"]


def _make_in_maps(x_temp, node_emb, w1_w, w1_b, w2_w, w2_b, wp_w, wp_b):
    x = np.ascontiguousarray(np.asarray(x_temp, dtype=np.float32))
    emb = np.ascontiguousarray(np.asarray(node_emb, dtype=np.float32))
    w1w = np.asarray(w1_w, dtype=np.float32)
    w2w = np.asarray(w2_w, dtype=np.float32)
    wpw = np.asarray(wp_w, dtype=np.float32)

    xt_full = np.ascontiguousarray(x.transpose(0, 2, 1)).reshape(B * D, N)
    embt = np.ascontiguousarray(emb.T)
    w1t = np.ascontiguousarray(w1w.T)
    w2t = np.ascontiguousarray(w2w.T)
    w1b_c = np.ascontiguousarray(np.asarray(w1_b, np.float32).reshape(D, 1))
    w2b_c = np.ascontiguousarray(np.asarray(w2_b, np.float32).reshape(D, 1))
    wp_col = wpw.reshape(D, 1)
    wp2 = np.ascontiguousarray(np.vstack([wp_col, wp_col]))

    in_maps = []
    for c in range(NCORES):
        rows = slice(ROWS * c, ROWS * (c + 1))
        in_maps.append(
            {
                "xt": xt_full,
                "xt_rows": np.ascontiguousarray(xt_full[:, rows]),
                "embt": embt,
                "embt_rows": np.ascontiguousarray(embt[:, rows]),
                "w1t": w1t,
                "w2t": w2t,
                "w1b": w1b_c,
                "w2b": w2b_c,
                "wp2": wp2,
                "ident": np.eye(128, dtype=np.float32),
            }
        )
    return in_maps


def kernel(**inputs):
    nc = _get_nc()
    in_maps = _make_in_maps(**inputs)
    res = run_bass_kernel_spmd(nc, in_maps, list(range(NCORES)), **_NC_CACHE.get("run_kwargs", {}))
    _NC_CACHE["last_result"] = res
    outs = [res.results[c]["out"].reshape(B, ROWS, N) for c in range(NCORES)]
    return np.concatenate(outs, axis=1)


# revision 14
# speedup vs baseline: 3.9621x; 3.9621x over previous
"""Trainium2 Bass kernel for nn_DynamicGraphLearner.

Computes, for full inputs (B=16, N=2048, D=64):
    adj_base = relu((emb @ w1.T + b1) @ (emb @ w2.T + b2).T)          [N, N]
    out      = softmax(adj_base + x xT + (v_i - v_j), axis=-1)        [B, N, N]
with v = x @ wp.T + wp_b.

Key algebraic simplifications (softmax is invariant to per-row shifts):
  * the +v_i term and the wp_b constant cancel entirely;
  * the -v_j term is linear in x_j, so it folds into the Gram matmul:
        logits_ij = adj_base_ij + (x_i - wp) . x_j
    i.e. the matmul lhs is simply (xT - wp) -- no separate bias pass.

Sharding: rows (the softmax i axis) are split 8 ways, 256 rows per core;
every core handles all 16 batches for its row slice, so the softmax stays
core-local and no collectives are needed.  adj_base is computed per core
only for its own 256-row slice.

Host-side marshaling: x_temp is passed transposed (xT [B, D, N] -> [B*D, N])
so the device reads clean contiguous 1 MiB tiles; two batches are packed per
SBUF tile (partitions 0:64 = batch 2q, 64:128 = batch 2q+1) and computed as
independent K=64 matmuls at partition bases 0 and 64.

Per [128, 2048] output tile:
  PE : 4 matmuls (512-col chunks) -> PSUM (dyn - v_j term), then 4
       identity-matmuls accumulating adj_base into the same PSUM banks
       (tensor_tensor_reduce is avoided: it wedges the device runtime)
  DVE: reduce_max over the PSUM tile (negated)
  ACT: exp(psum + (-max)) PSUM->SBUF with fused row-sum accumulation
  DVE: reciprocal, renormalize (tensor_scalar 2x mode)
  DMA: store 1 MiB contiguous
"""

import sys

import numpy as np

try:
    import concourse.bass as bass
except ImportError:  # environment provides concourse via /opt/trn_rl_repo
    sys.path.insert(0, "/opt/trn_rl_repo")
    import concourse.bass as bass

import concourse.tile as tile
from concourse import bacc, mybir
from concourse.bass_utils import run_bass_kernel_spmd

NCORES = 8
B, N, D = 16, 2048, 64
ROWS = N // NCORES  # 256 rows per core
FP = mybir.dt.float32

_NC_CACHE = {}


def _build_nc(reps=1):
    # reps>1 repeats the main loop (same outputs, idempotent) — used only by
    # the benchmark harness to amortize per-dispatch overhead out of timings.
    nc = bacc.Bacc(None)

    xt = nc.dram_tensor("xt", [B * D, N], FP, kind="ExternalInput")
    xt_rows = nc.dram_tensor("xt_rows", [B * D, ROWS], FP, kind="ExternalInput")
    embt = nc.dram_tensor("embt", [D, N], FP, kind="ExternalInput")
    embt_rows = nc.dram_tensor("embt_rows", [D, ROWS], FP, kind="ExternalInput")
    w1t = nc.dram_tensor("w1t", [D, D], FP, kind="ExternalInput")
    w2t = nc.dram_tensor("w2t", [D, D], FP, kind="ExternalInput")
    w1b = nc.dram_tensor("w1b", [D, 1], FP, kind="ExternalInput")
    w2b = nc.dram_tensor("w2b", [D, 1], FP, kind="ExternalInput")
    wp2 = nc.dram_tensor("wp2", [128, 1], FP, kind="ExternalInput")
    ident = nc.dram_tensor("ident", [128, 128], FP, kind="ExternalInput")
    out = nc.dram_tensor("out", [B * ROWS, N], FP, kind="ExternalOutput")

    Exp = mybir.ActivationFunctionType.Exp
    Relu = mybir.ActivationFunctionType.Relu
    Alu = mybir.AluOpType

    with tile.TileContext(nc) as tc:
        with (
            tc.tile_pool(name="const", bufs=1) as cpool,
            tc.tile_pool(name="ps", bufs=2, space="PSUM") as ps,
            tc.tile_pool(name="xp", bufs=3) as xpool,
            tc.tile_pool(name="op", bufs=3) as opool,
            tc.tile_pool(name="st", bufs=4) as spool,
        ):
            # ---- constants ----
            embt_sb = cpool.tile([D, N], FP)
            nc.scalar.dma_start(embt_sb[:], embt[:])
            embtr_sb = cpool.tile([D, ROWS], FP)
            nc.scalar.dma_start(embtr_sb[:], embt_rows[:])
            w1t_sb = cpool.tile([D, D], FP)
            nc.scalar.dma_start(w1t_sb[:], w1t[:])
            w2t_sb = cpool.tile([D, D], FP)
            nc.scalar.dma_start(w2t_sb[:], w2t[:])
            w1b_sb = cpool.tile([D, 1], FP)
            nc.scalar.dma_start(w1b_sb[:], w1b[:])
            w2b_sb = cpool.tile([D, 1], FP)
            nc.scalar.dma_start(w2b_sb[:], w2b[:])
            wp2_sb = cpool.tile([128, 1], FP)
            nc.scalar.dma_start(wp2_sb[:], wp2[:])
            id_sb = cpool.tile([128, 128], FP)
            nc.scalar.dma_start(id_sb[:], ident[:])

            # ---- node_1cT [64, 256] = w1 @ embT[:, rows] + b1 ----
            p1 = ps.tile([128, 2048], FP, tag="pm")
            nc.tensor.matmul(p1[0:D, 0:ROWS], w1t_sb[:], embtr_sb[:], start=True, stop=True)
            n1t_sb = cpool.tile([D, ROWS], FP)
            nc.vector.tensor_scalar_add(n1t_sb[:], p1[0:D, 0:ROWS], w1b_sb[:])

            # ---- node_2T [64, 2048] = w2 @ embT + b2 ----
            n2t_sb = cpool.tile([D, N], FP)
            p2 = ps.tile([128, 2048], FP, tag="pm")
            for c in range(4):
                nc.tensor.matmul(
                    p2[0:D, c * 512 : (c + 1) * 512],
                    w2t_sb[:],
                    embt_sb[:, c * 512 : (c + 1) * 512],
                    start=True,
                    stop=True,
                )
            nc.vector.tensor_scalar_add(n2t_sb[:], p2[0:D, :], w2b_sb[:])

            # ---- adj_base rows slice: 2 tiles of [128, 2048], relu'd ----
            adj_sb = []
            for rt in range(2):
                a = cpool.tile([128, N], FP, name=f"adj{rt}")
                adj_sb.append(a)
            for rt in range(2):
                pa = ps.tile([128, 2048], FP, tag="pm", name=f"pa_{rt}")
                for c in range(4):
                    nc.tensor.matmul(
                        pa[:, c * 512 : (c + 1) * 512],
                        n1t_sb[:, rt * 128 : (rt + 1) * 128],
                        n2t_sb[:, c * 512 : (c + 1) * 512],
                        start=True,
                        stop=True,
                    )
                nc.scalar.activation(adj_sb[rt][:], pa[:], Relu)

            # ---- main loop: 8 batch-pairs x 2 batches x 2 row tiles ----
            for rep, q in [(r, qq) for r in range(reps) for qq in range(NCORES)]:
                # packed pair of batches: partitions 0:64 = batch 2q, 64:128 = 2q+1
                xt_sb = xpool.tile([128, N], FP, tag="xt", name=f"xt{rep}_{q}")
                nc.scalar.dma_start(xt_sb[:], xt[q * 128 : (q + 1) * 128, :])
                xtr_sb = xpool.tile([128, ROWS], FP, tag="xtr", name=f"xtr{rep}_{q}")
                nc.scalar.dma_start(xtr_sb[:], xt_rows[q * 128 : (q + 1) * 128, :])
                # lhs = xT(rows) - wp  (folds the -v_j term into the matmul)
                lhs = xpool.tile([128, ROWS], FP, tag="lhs", name=f"lhs{rep}_{q}")
                nc.vector.tensor_scalar(
                    lhs[:], xtr_sb[:], wp2_sb[:], None, op0=Alu.subtract
                )

                for sb in range(2):  # sub-batch within the pair
                    base = D * sb
                    b = 2 * q + sb
                    for rt in range(2):  # row tile within the 256-row slice
                        o_sb = opool.tile([128, N], FP, tag="o", name=f"o{rep}_{q}_{sb}_{rt}")
                        nmax = spool.tile([128, 1], FP, tag="nm", name=f"nm{rep}_{b}_{rt}")
                        ssum = spool.tile([128, 1], FP, tag="ss", name=f"ss{rep}_{b}_{rt}")
                        rcp = spool.tile([128, 1], FP, tag="rc", name=f"rc{rep}_{b}_{rt}")

                        pm = ps.tile([128, N], FP, tag="pm", name=f"pm{rep}_{b}_{rt}")
                        for c in range(4):
                            nc.tensor.matmul(
                                pm[:, c * 512 : (c + 1) * 512],
                                lhs[base : base + D, rt * 128 : (rt + 1) * 128],
                                xt_sb[base : base + D, c * 512 : (c + 1) * 512],
                                start=True,
                                stop=False,
                            )
                        # accumulate adj_base into the same PSUM banks via
                        # identity matmul (I.T @ adj = adj, has_written add)
                        for c in range(4):
                            nc.tensor.matmul(
                                pm[:, c * 512 : (c + 1) * 512],
                                id_sb[:],
                                adj_sb[rt][:, c * 512 : (c + 1) * 512],
                                start=False,
                                stop=True,
                            )
                        # negated row max of the full logit tile
                        nc.vector.tensor_reduce(
                            nmax[:], pm[:], axis=mybir.AxisListType.X, op=Alu.max,
                            negate=True,
                        )
                        # exp(psum - max) -> SBUF, with fused row-sum
                        nc.scalar.activation(
                            o_sb[:], pm[:], Exp, bias=nmax[:], scale=1.0,
                            accum_out=ssum[:],
                        )
                        nc.vector.reciprocal(rcp[:], ssum[:])
                        nc.vector.tensor_scalar_mul(o_sb[:], o_sb[:], rcp[:])
                        row0 = b * ROWS + rt * 128
                        nc.sync.dma_start(out[row0 : row0 + 128, :], o_sb[:])

    nc.finalize()
    return nc


def _get_nc():
    if "nc" not in _NC_CACHE:
        _NC_CACHE["nc"] = _build_nc()
    return _NC_CACHE["nc"]


def _make_in_maps(x_temp, node_emb, w1_w, w1_b, w2_w, w2_b, wp_w, wp_b):
    x = np.ascontiguousarray(np.asarray(x_temp, dtype=np.float32))
    emb = np.ascontiguousarray(np.asarray(node_emb, dtype=np.float32))
    w1w = np.asarray(w1_w, dtype=np.float32)
    w2w = np.asarray(w2_w, dtype=np.float32)
    wpw = np.asarray(wp_w, dtype=np.float32)

    xt_full = np.ascontiguousarray(x.transpose(0, 2, 1)).reshape(B * D, N)
    embt = np.ascontiguousarray(emb.T)
    w1t = np.ascontiguousarray(w1w.T)
    w2t = np.ascontiguousarray(w2w.T)
    w1b_c = np.ascontiguousarray(np.asarray(w1_b, np.float32).reshape(D, 1))
    w2b_c = np.ascontiguousarray(np.asarray(w2_b, np.float32).reshape(D, 1))
    wp_col = wpw.reshape(D, 1)
    wp2 = np.ascontiguousarray(np.vstack([wp_col, wp_col]))

    in_maps = []
    for c in range(NCORES):
        rows = slice(ROWS * c, ROWS * (c + 1))
        in_maps.append(
            {
                "xt": xt_full,
                "xt_rows": np.ascontiguousarray(xt_full[:, rows]),
                "embt": embt,
                "embt_rows": np.ascontiguousarray(embt[:, rows]),
                "w1t": w1t,
                "w2t": w2t,
                "w1b": w1b_c,
                "w2b": w2b_c,
                "wp2": wp2,
                "ident": np.eye(128, dtype=np.float32),
            }
        )
    return in_maps


def kernel(**inputs):
    nc = _get_nc()
    in_maps = _make_in_maps(**inputs)
    res = run_bass_kernel_spmd(nc, in_maps, list(range(NCORES)), **_NC_CACHE.get("run_kwargs", {}))
    _NC_CACHE["last_result"] = res
    outs = [res.results[c]["out"].reshape(B, ROWS, N) for c in range(NCORES)]
    return np.concatenate(outs, axis=1)
